# revision 11
# baseline (speedup 1.0000x reference)
"""Trainium2 Bass kernel for ConvScoreSSREM loss.

Computes, for B=16384 rows (data-parallel, 2048 rows per NeuronCore x 8):
    cm        = contexts @ mat_M                    [B, E]
    scores_k  = sum_e cm[b,e] * res_k[b,e]          k in 0..4
    out[b]    = log_softmax(scores)[:, 0]

All inputs are downcast to fp16 on the host (rel err ~4e-4 vs the fp32
reference, well under the 2e-2 gate); fp16 halves HBM traffic, streams the
PE at 1 cycle/row, and unlocks the DVE 2x/4x 16-bit perf modes.

contexts is also pre-transposed/blocked on the host into
    ctxb[g, p, (a k j)] = ctx[(2g+a)*128 + j, 128k + p]
so each SBUF tile arrives transposed ([e, b] chunks, one contiguous
4KB-per-partition DMA per 256-row group) and the PE does pure matmul work --
no identity-transpose passes, no PSUM drain copies.  mat_M is likewise
host-blocked to  mb[p, (k e)] = M[128k + p, e]  and DMA'd straight into its
resident SBUF tile (no staging/convert copies).

Per 128-row tile: 16 fp16 matmuls accumulate cm[128b, 1024e'] into a 2-bank
PSUM tile (4 tiles cycle through all 8 banks); ACT copies cm to fp16 SBUF
(frees the bank, keeps DVE off PSUM and in 16-bit perf mode); 5 fused DVE
multiply+accumulate ops produce the scores; one log-softmax tail and a
single DMA out.
"""

import numpy as np

import concourse.bacc as bacc
import concourse.mybir as mybir
import concourse.tile as tile
from concourse.bass_utils import run_bass_kernel_spmd

B = 16384
E = 1024
NCORES = 8
BS = B // NCORES  # 2048 rows per core
P = 128
NT = BS // P      # 16 row-tiles per core
NG = NT // 2      # 8 pair-groups
KC = E // P       # 8 contraction chunks
NK = 5            # number of res tensors
NHALF = 512       # matmul moving free-dim (one PSUM bank of fp32)

F32 = mybir.dt.float32
DT = mybir.dt.float16
NPDT = np.float16

RES_NAMES = ["res0", "res1", "res2", "res3", "res4"]


DEFAULT_OPTS = dict(
    ctx_bufs=3,
    res_bufs=3,    # [P, NK, 2, E] per pair-group
    cms_bufs=3,
    prod_bufs=2,
    junk_bufs=2,
    pcm_bufs=4,    # 4 x [128,1024]f32 = all 8 PSUM banks
    pool_k3_mod=2,  # Pool also multiplies k=3 when t % mod == 0 (k=4 always)
    act_k1_mod=2,   # ACT reduces k=0 always, k=1 when t % mod == 1; rest DVE @4x
)


def build_nc(repeat=1, internal_inputs=False, opts=None):
    """Build + compile the single-core Bass program (same program on all 8 cores).

    repeat>1 replays the steady-state compute loop; internal_inputs=True reads
    contexts/res from internal DRAM scratch instead of ExternalInputs (both are
    timing aids only)."""
    nc = bacc.Bacc("TRN2", debug=False, enable_asserts=False, num_devices=NCORES)

    kind = "Internal" if internal_inputs else "ExternalInput"
    sfx = "_i" if internal_inputs else ""
    ctx_d = nc.dram_tensor("ctxb" + sfx, (NG, P, 2 * KC * P), DT, kind=kind)
    res_d = [nc.dram_tensor(n + sfx, (BS, E), DT, kind=kind) for n in RES_NAMES]
    m_d = nc.dram_tensor("mat_Mb", (P, KC * E), DT, kind="ExternalInput")
    out_d = nc.dram_tensor("out", (BS,), F32, kind="ExternalOutput")

    o = dict(DEFAULT_OPTS)
    if opts:
        o.update(opts)
    with tile.TileContext(nc) as tc:
        _body(nc, tc, ctx_d.ap(), [r.ap() for r in res_d], m_d.ap(), out_d.ap(),
              repeat=repeat, o=o)

    nc.compile()
    return nc


def _body(nc, tc, ctx_d, res_d, m_d, out_d, repeat=1, o=None):
    o = o or DEFAULT_OPTS
    # res DRAM view with pair-groups split out: row (g*2 + a)*128 + p.
    res_g = [r.rearrange("(g a p) e -> g p a e", a=2, p=P) for r in res_d]

    with (
        tc.tile_pool(name="mpool", bufs=1) as mpool,
        tc.tile_pool(name="ctxp", bufs=o["ctx_bufs"]) as ctxp,
        tc.tile_pool(name="resp", bufs=o["res_bufs"]) as resp,
        tc.tile_pool(name="cmsb", bufs=o["cms_bufs"]) as cmsb,
        tc.tile_pool(name="prodp", bufs=o["prod_bufs"]) as prodp,
        tc.tile_pool(name="junkp", bufs=o["junk_bufs"]) as junkp,
        tc.tile_pool(name="dumpp", bufs=o["junk_bufs"]) as dumpp,
        tc.tile_pool(name="smallp", bufs=1) as smallp,
        tc.tile_pool(name="pcm", bufs=o["pcm_bufs"], space="PSUM") as pcm,
    ):
        # mat_M resident: m_sbr[p, k, :] = M[k*128 + p, :], DMA'd directly
        # (host pre-blocked).  Chunked per k across both rings so the first
        # matmuls only wait on chunk 0.
        m_sbr = mpool.tile([P, KC, E], DT)
        for k in range(KC):
            eng = nc.sync if k % 2 == 0 else nc.scalar
            eng.dma_start(m_sbr[:, k, :], m_d[:, k * E : (k + 1) * E])

        scores = smallp.tile([P, NT, NK], F32)

        for _rep in range(repeat):
            for g in range(NG):
                ctx_t = ctxp.tile([P, 2, KC, P], DT, tag="ctx")
                nc.scalar.dma_start(
                    ctx_t[:].rearrange("p a k j -> p (a k j)"), ctx_d[g]
                )
                res5 = resp.tile([P, NK, 2, E], DT, tag="res")
                for k in range(NK):
                    eng = nc.sync if k < 3 else nc.scalar
                    eng.dma_start(res5[:, k, :, :], res_g[k][g])

                for a in range(2):
                    t = 2 * g + a
                    # cm[128b, 1024e'] accumulated over 8 contraction chunks
                    cm = pcm.tile([P, E], F32, tag="cm")
                    for k in range(KC):
                        for h in range(2):
                            nc.tensor.matmul(
                                cm[:, h * NHALF : (h + 1) * NHALF],
                                ctx_t[:, a, k, :],
                                m_sbr[:, k, h * NHALF : (h + 1) * NHALF],
                                start=(k == 0),
                                stop=(k == KC - 1),
                            )

                    # PSUM -> SBUF fp16: frees the bank, keeps the dot ops in
                    # 16-bit perf modes off PSUM
                    cm_s = cmsb.tile([P, E], DT, tag="cms")
                    nc.scalar.copy(cm_s[:], cm[:])

                    # scores[:, t, k] = sum_e' cm * res_k.  The fused
                    # mult+accum op (scalar_tensor_tensor) has no 16-bit fast
                    # mode (1 elem/cycle = the old wall), so split it:
                    #   multiply: TensorTensor @2x on DVE / Pool(gpsimd)
                    #   reduce:   tensor_scalar+accum @4x on DVE / ACT accum
                    kd = 3 if t % o["pool_k3_mod"] == 0 else 4  # DVE mult k<kd
                    prod = prodp.tile([P, NK, E], DT, tag="prod")
                    cmb = cm_s[:, None, :]
                    nc.vector.tensor_tensor(
                        out=prod[:, :kd, :],
                        in0=cmb.broadcast_to([P, kd, E]),
                        in1=res5[:, :kd, a, :],
                        op=mybir.AluOpType.mult,
                    )
                    nc.gpsimd.tensor_tensor(
                        out=prod[:, kd:, :],
                        in0=cmb.broadcast_to([P, NK - kd, E]),
                        in1=res5[:, kd:, a, :],
                        op=mybir.AluOpType.mult,
                    )
                    act_ks = (0, 1) if t % o["act_k1_mod"] == 1 else (0,)
                    for k in range(NK):
                        if k in act_ks:
                            dump = dumpp.tile([P, E], DT, tag="dump")
                            nc.scalar.activation(
                                dump[:],
                                prod[:, k, :],
                                mybir.ActivationFunctionType.Copy,
                                accum_out=scores[:, t, k : k + 1],
                            )
                        else:
                            junk = junkp.tile([P, E], DT, tag="junk")
                            nc.vector.tensor_scalar(
                                out=junk[:],
                                in0=prod[:, k, :],
                                scalar1=1.0,
                                scalar2=0.0,
                                op0=mybir.AluOpType.mult,
                                op1=mybir.AluOpType.add,
                                accum_out=scores[:, t, k : k + 1],
                            )

        # ---- log-softmax tail over [P, NT, NK] ----
        mx = smallp.tile([P, NT], F32)
        nc.vector.tensor_reduce(
            out=mx[:], in_=scores[:], axis=mybir.AxisListType.X, op=mybir.AluOpType.max
        )
        d = smallp.tile([P, NT, NK], F32)
        mx_b = mx[:, :, None].broadcast_to([P, NT, NK])
        nc.vector.tensor_tensor(
            out=d[:], in0=scores[:], in1=mx_b, op=mybir.AluOpType.subtract
        )
        ex = smallp.tile([P, NT, NK], F32)
        nc.scalar.activation(ex[:], d[:], mybir.ActivationFunctionType.Exp)
        ssum = smallp.tile([P, NT], F32)
        nc.vector.tensor_reduce(
            out=ssum[:], in_=ex[:], axis=mybir.AxisListType.X, op=mybir.AluOpType.add
        )
        lse = smallp.tile([P, NT], F32)
        nc.scalar.activation(lse[:], ssum[:], mybir.ActivationFunctionType.Ln)
        outsb = smallp.tile([P, NT], F32)
        nc.vector.tensor_sub(outsb[:], d[:, :, 0], lse[:])

        nc.sync.dma_start(out_d.rearrange("(t p) -> p t", p=P), outsb[:])


_NC_CACHE = None


def _get_nc():
    global _NC_CACHE
    if _NC_CACHE is None:
        _NC_CACHE = build_nc()
    return _NC_CACHE


def timing_in_maps():
    """External inputs for an internal_inputs=True timing build."""
    return {"mat_Mb": np.zeros((P, KC * E), NPDT)}


def _block_ctx(c16):
    """[BS, E] fp16 -> [NG, P, 2*KC*P] with [g,p,a,k,j] = ctx[(2g+a)*128+j, 128k+p]."""
    cb = c16.reshape(NG, 2, P, KC, P)      # [g, a, j, k, p]
    cb = cb.transpose(0, 4, 1, 3, 2)       # [g, p, a, k, j]
    return np.ascontiguousarray(cb).reshape(NG, P, 2 * KC * P)


def make_in_maps(contexts, res_pos, res_neg1, res_neg2, res_neg3, res_neg4, mat_M):
    ctx16 = np.asarray(contexts).astype(NPDT)
    ress = [
        np.asarray(r).astype(NPDT)
        for r in (res_pos, res_neg1, res_neg2, res_neg3, res_neg4)
    ]
    m16 = np.asarray(mat_M).astype(NPDT)
    mb = np.ascontiguousarray(
        m16.reshape(KC, P, E).transpose(1, 0, 2)
    ).reshape(P, KC * E)
    in_maps = []
    for c in range(NCORES):
        sl = slice(c * BS, (c + 1) * BS)
        m = {"ctxb": _block_ctx(ctx16[sl]), "mat_Mb": mb}
        for name, r in zip(RES_NAMES, ress):
            m[name] = r[sl]
        in_maps.append(m)
    return in_maps


def kernel(contexts, res_pos, res_neg1, res_neg2, res_neg3, res_neg4, mat_M):
    nc = _get_nc()
    in_maps = make_in_maps(
        contexts, res_pos, res_neg1, res_neg2, res_neg3, res_neg4, mat_M
    )
    res = run_bass_kernel_spmd(nc, in_maps, core_ids=list(range(NCORES)))
    out = np.concatenate([res.results[c]["out"] for c in range(NCORES)])
    return out.astype(np.float32, copy=False)
